# revision 13
# baseline (speedup 1.0000x reference)
"""AutoCompleteDecoderModel (LSTM enc-dec + CE loss) on 8 Trainium2 cores.

Strategy (hardcoded for B=256, S=512, H=512, V=128):
 - Data-parallel: 8 cores x 32 samples.
 - Per core, per time step the recurrent matmul is computed transposed:
   gates.T [2048, 32] as 16 M-tiles of [128, 32] packed into two PSUM
   tiles psA/psB (col 32*m+j = gate-dim 128*m+p of sample j).  Each
   PSUM tile is a FULL 2KB bank and holds ONE accumulation group per
   step: the first matmul into the bank has start=True (lazy-zeroes the
   whole 2KB zero region), everything else start=False, and only the
   bank's last matmul sets stop=True.  lhsT = weight tile [128,128] in
   fp8e4m3 scaled by WSCALE (FWL-friendly), rhs = hT bf16 / one-hot fp8.
 - Gate m-tile order is torch-native [i | f | g | o]: bank A = [i|f],
   bank B = [g|o].  The chain uses DIRECT Sigmoid/Tanh LUTs (both live
   in the 'sigmoid_and_others' activation table), descaling by 1/WSCALE
   via the ACT scale; gate combines are plain tensor_mul/tensor_add in
   bf16 (2x DVE mode), minimizing the serial critical path:
     sig_if [128,256] -> u2 = sig_f*c ; tanh_g -> u1 = sig_i*tg ;
     c' = u1+u2 ; sig_o (off-path) ; tanh_c ; hT = sig_o*tanh_c.
   Steady state is ~2.74us/step, bound by the cross-engine dependency
   chain (2 ACT hops + 3 DVE hops + bank-A matmul head), not by any
   engine's throughput.
 - One-hot input projection: host precomputes one-hot streams
   [128, S*32] fp8; per step it is the first K-tile of each PSUM
   accumulation group (biases folded into W_ih columns).
 - Decoder: logits.T [128, 32] = proj_W @ h via 4 k-MMs into PSUM
   (deferred one step so they fill the chain stall), copied to an SBUF
   stack (bf16) and DMA'd out raw; the HOST computes log-softmax/NLL/
   masked mean in f64 (no Exp table needed on device).
 - Preamble: weight DMAs split across the SP/ACT hwdge queues in
   need-order; ~250 junk matmuls warm the PE out of its low-power
   state while the DMAs are in flight.
"""

import os
import sys

import numpy as np

if "/opt/trn_rl_repo" not in sys.path:
    sys.path.insert(0, "/opt/trn_rl_repo")

B, S, H, V = 256, 512, 512, 128
NCORES = 8
BS = B // NCORES  # 32 samples per core
U = 64  # steps per hw-loop iteration
WSCALE = 256.0  # fp8 weight scaling: weights stored *WSCALE, descaled in ACT

_CACHE = {}


def _prep_weights(W_ih, W_hh, b_ih, b_hh):
    """Fold biases into W_ih (one-hot input => bias adds once per step),
    scale by WSCALE, quantize to fp8e4m3, and tile for the kernel
    layouts.  Gate order stays torch-native [i|f|g|o]."""
    import ml_dtypes

    f8 = ml_dtypes.float8_e4m3
    Wi = WSCALE * (np.asarray(W_ih, np.float32) + (np.asarray(b_ih, np.float32)
          + np.asarray(b_hh, np.float32))[:, None])      # [2048, 128]
    Wh = WSCALE * np.asarray(W_hh, np.float32)            # [2048, 512]
    # input proj: lhsT tiles = W_ih'.T [V=128, 2048]
    wih_t = np.ascontiguousarray(Wi.T).astype(f8)
    # recurrent: tiles [k, m] = W_hh.T[128k:128k+128, 128m:128m+128]
    # packed [128, 64*128] with col (k*16+m)*128 + c
    Wt = np.ascontiguousarray(Wh.T)  # [512, 2048]
    whh_t = (Wt.reshape(4, 128, 16, 128).transpose(1, 0, 2, 3)
             .reshape(128, 8192).astype(f8))
    return np.ascontiguousarray(wih_t), np.ascontiguousarray(whh_t)


def _onehot_stream(idx):
    """idx [BS, S] int -> [128, S*32] fp8, col t*32+j = (idx[j,t]==v)."""
    import ml_dtypes
    oh = (np.arange(V, dtype=np.int32)[:, None, None]
          == np.asarray(idx, np.int32).T[None, :, :])  # [V, S, BS]
    return np.ascontiguousarray(
        oh.reshape(V, -1).astype(ml_dtypes.float8_e4m3))


def _build_module(n_steps, unrolled=False):
    """Build + compile the Bass/Tile module (same program on all cores).
    unrolled=True replaces the hw For_i loops with static python loops
    (for cost-model simulation of small n_steps)."""
    import concourse.bacc as bacc
    import concourse.bass as bass
    import concourse.mybir as mybir
    import concourse.tile as tile

    f32 = mybir.dt.float32
    bf16 = mybir.dt.bfloat16
    AF = mybir.ActivationFunctionType
    PE = mybir.EngineType.PE

    Ueff = min(U, n_steps)
    n_iters = n_steps // Ueff

    nc = bacc.Bacc("TRN2", target_bir_lowering=False, debug=False,
                   num_devices=NCORES)

    # ---- DRAM I/O ----
    f8 = mybir.dt.float8e4
    d_enc_whh = nc.dram_tensor("enc_whh", [128, 8192], f8, kind="ExternalInput").ap()
    d_enc_wih = nc.dram_tensor("enc_wih", [128, 2048], f8, kind="ExternalInput").ap()
    d_dec_whh = nc.dram_tensor("dec_whh", [128, 8192], f8, kind="ExternalInput").ap()
    d_dec_wih = nc.dram_tensor("dec_wih", [128, 2048], f8, kind="ExternalInput").ap()
    d_projt = nc.dram_tensor("projt", [128, 512], f8, kind="ExternalInput").ap()
    d_enc_oh = nc.dram_tensor("enc_oh", [128, n_steps * BS], f8, kind="ExternalInput").ap()
    d_dec_oh = nc.dram_tensor("dec_oh", [128, n_steps * BS], f8, kind="ExternalInput").ap()
    d_logits = nc.dram_tensor("logits", [128, n_steps * BS], bf16, kind="ExternalOutput").ap()

    with tile.TileContext(nc) as tc:
        with (
            tc.tile_pool(name="const", bufs=1) as const_pool,
            tc.tile_pool(name="oh", bufs=2) as oh_pool,
            tc.tile_pool(name="gates", bufs=2, space="PSUM") as gatesA_pool,
            tc.tile_pool(name="gatesB", bufs=2, space="PSUM") as gatesB_pool,
            tc.tile_pool(name="logits", bufs=2, space="PSUM") as logits_pool,
            tc.tile_pool(name="work", bufs=3) as work_pool,
            tc.tile_pool(name="stack", bufs=2) as stack_pool,
        ):
            # persistent weights + state
            w_enc_hh = const_pool.tile([128, 8192], f8, tag="wehh")
            w_enc_ih = const_pool.tile([128, 2048], f8, tag="weih")
            w_dec_hh = const_pool.tile([128, 8192], f8, tag="wdhh")
            w_dec_ih = const_pool.tile([128, 2048], f8, tag="wdih")
            w_projt = const_pool.tile([128, 512], f8, tag="wpt")
            # hT double-buffer: slot p holds cols [p*128, p*128+128)
            hTb = const_pool.tile([128, 256], bf16, tag="hTb")
            cst = const_pool.tile([128, 128], bf16, tag="cst")

            nc.sync.dma_start(w_enc_ih[:], d_enc_wih)
            nc.scalar.dma_start(w_enc_hh[:], d_enc_whh)
            nc.scalar.dma_start(w_dec_hh[:], d_dec_whh)
            nc.scalar.dma_start(w_dec_ih[:], d_dec_wih)
            nc.scalar.dma_start(w_projt[:], d_projt)
            nc.vector.memset(hTb[:], 0.0)
            nc.vector.memset(cst[:], 0.0)
            # PE warmup: ~250 matmuls on junk data bring the PE out of its
            # low-power state while the weight DMAs are in flight, so step 0
            # runs at full clock.  Own PSUM bank, own group.
            warm = const_pool.tile([128, 128], bf16, tag="warm")
            nc.vector.memset(warm[:], 0.0)
            ps_w = logits_pool.tile([128, 512], f32, tag="psl")
            for r in range(250):
                nc.tensor.matmul(ps_w[:, 0:32], warm[:], warm[:, 0:32],
                                 start=(r == 0), stop=(r == 249))

            def _pslice(pair, m):
                psA, psB = pair
                t = psA if m < 8 else psB
                mm = m % 8
                return t[:, mm * 32:(mm + 1) * 32]

            def inproj(w_ih, xt):
                """Input projection (+folded bias) for one step: K-tile 0 of
                each PSUM accumulation group.  No dependence on hT, so these
                matmuls fill the PE while the previous step's chain runs."""
                psA = gatesA_pool.tile([128, 512], f32, tag="psA")
                psB = gatesB_pool.tile([128, 512], f32, tag="psB")
                pair = (psA, psB)
                # one accumulation group per PSUM bank: the first matmul
                # into a bank has start=True (lazy-zeroes the whole 2KB
                # zero region); every other matmul start=False (overwrite
                # pending-zero bytes / accumulate written ones).
                for m in range(16):
                    nc.tensor.matmul(_pslice(pair, m),
                                     w_ih[:, m * 128:(m + 1) * 128],
                                     xt, start=(m % 8 == 0), stop=False)
                return pair

            def k_mms(w_hh, pair, u):
                # recurrent K-tiles reading hT(u-1) (slot (u+1)%2);
                # m-major so PSUM slices finish in order
                hs = ((u + 1) % 2) * 128
                for m in range(16):
                    for k in range(4):
                        # stop only on the LAST matmul of each bank's group
                        nc.tensor.matmul(
                            _pslice(pair, m),
                            w_hh[:, (k * 16 + m) * 128:(k * 16 + m + 1) * 128],
                            hTb[:, hs + k * 32:hs + (k + 1) * 32],
                            start=False, stop=(k == 3 and m % 8 == 7))

            def chain(pair, u):
                """Gate activations + state update, direct sigmoid/tanh.
                bank A cols: [i 0:128 | f 128:256]; bank B: [g 0:128 | o 128:256]
                """
                psA, psB = pair
                hs = (u % 2) * 128
                sif = work_pool.tile([128, 256], bf16, tag="sif")
                nc.scalar.activation(sif[:], psA[:, 0:256], AF.Sigmoid, scale=1.0 / WSCALE)
                tg = work_pool.tile([128, 128], bf16, tag="tg")
                nc.scalar.activation(tg[:], psB[:, 0:128], AF.Tanh, scale=1.0 / WSCALE)
                so = work_pool.tile([128, 128], bf16, tag="so")
                nc.scalar.activation(so[:], psB[:, 128:256], AF.Sigmoid, scale=1.0 / WSCALE)
                u2 = work_pool.tile([128, 128], bf16, tag="u2")
                nc.vector.tensor_mul(u2[:], sif[:, 128:256], cst[:])
                u1 = work_pool.tile([128, 128], bf16, tag="u1")
                nc.vector.tensor_mul(u1[:], sif[:, 0:128], tg[:])
                nc.vector.tensor_add(cst[:], u1[:], u2[:])
                tc2 = work_pool.tile([128, 128], bf16, tag="tc2")
                nc.scalar.activation(tc2[:], cst[:], AF.Tanh)
                nc.vector.tensor_mul(hTb[:, hs:hs + 128], so[:], tc2[:])

            def proj_mms(u):
                """Projection matmuls for step u's logits (reads hT slot u%2)."""
                hs = (u % 2) * 128
                ps_l = logits_pool.tile([128, 512], f32, tag="psl")
                for k in range(4):
                    nc.tensor.matmul(ps_l[:, 0:32],
                                     w_projt[:, k * 128:(k + 1) * 128],
                                     hTb[:, hs + k * 32:hs + (k + 1) * 32],
                                     start=(k == 0), stop=(k == 3))
                return ps_l

            def enc_body(i):
                oh = oh_pool.tile([128, Ueff * BS], f8, tag="oh")
                nc.sync.dma_start(oh[:], d_enc_oh[:, bass.ts(i, Ueff * BS)])
                # inproj(u+1) emitted between k_mms(u) and chain(u): PE
                # runs it while the chain produces hT(u).
                ps = inproj(w_enc_ih, oh[:, 0:BS])
                for u in range(Ueff):
                    k_mms(w_enc_hh, ps, u)
                    if u + 1 < Ueff:
                        ps_n = inproj(w_enc_ih, oh[:, (u + 1) * BS:(u + 2) * BS])
                    chain(ps, u)
                    ps = ps_n

            def dec_body(i):
                oh = oh_pool.tile([128, Ueff * BS], f8, tag="oh")
                nc.sync.dma_start(oh[:], d_dec_oh[:, bass.ts(i, Ueff * BS)])
                stack = stack_pool.tile([128, Ueff * BS], bf16, tag="stk")
                ps = inproj(w_dec_ih, oh[:, 0:BS])
                pend = None  # (ps_l, u) logits awaiting copy to stack
                for u in range(Ueff):
                    k_mms(w_dec_hh, ps, u)
                    if u + 1 < Ueff:
                        ps_n = inproj(w_dec_ih, oh[:, (u + 1) * BS:(u + 2) * BS])
                    if u > 0:
                        # proj for the PREVIOUS step: runs on PE during
                        # chain(u)'s stall; hT slot (u-1)%2 is still live.
                        pend = (proj_mms(u - 1), u - 1)
                    chain(ps, u)
                    if pend is not None:
                        ps_l, pu = pend
                        nc.vector.tensor_copy(stack[:, pu * BS:(pu + 1) * BS],
                                              ps_l[:, 0:32])
                        pend = None
                    ps = ps_n
                ps_l = proj_mms(Ueff - 1)
                nc.vector.tensor_copy(stack[:, (Ueff - 1) * BS:Ueff * BS],
                                      ps_l[:, 0:32])
                nc.sync.dma_start(d_logits[:, bass.ts(i, Ueff * BS)], stack[:])

            if unrolled:
                for i in range(n_iters):
                    enc_body(i)
                for i in range(n_iters):
                    dec_body(i)
            else:
                with tc.For_i(0, n_iters, 1, hint_engines=(PE,), name="enc") as i:
                    enc_body(i)
                with tc.For_i(0, n_iters, 1, hint_engines=(PE,), name="dec") as i:
                    dec_body(i)

    nc.compile()
    return nc


def _run(inputs, n_steps=S, trace=False):
    from concourse import bass_utils

    key = n_steps
    if key not in _CACHE:
        _CACHE[key] = _build_module(n_steps)
    nc = _CACHE[key]

    enc_wih, enc_whh = _prep_weights(inputs["enc_W_ih"], inputs["enc_W_hh"],
                                     inputs["enc_b_ih"], inputs["enc_b_hh"])
    dec_wih, dec_whh = _prep_weights(inputs["dec_W_ih"], inputs["dec_W_hh"],
                                     inputs["dec_b_ih"], inputs["dec_b_hh"])
    import ml_dtypes
    projW = WSCALE * np.asarray(inputs["proj_W"], np.float32)  # [128, 512]
    projt = (np.ascontiguousarray(projW.T).reshape(4, 128, 128)
             .transpose(1, 0, 2).reshape(128, 512).astype(ml_dtypes.float8_e4m3))

    C_idx = np.asarray(inputs["C_idx"])[:, :n_steps]
    E = np.asarray(inputs["E"])
    Etgt = E[:, :n_steps]

    in_maps = []
    for c in range(NCORES):
        sl = slice(c * BS, (c + 1) * BS)
        in_maps.append({
            "enc_whh": enc_whh, "enc_wih": enc_wih,
            "dec_whh": dec_whh, "dec_wih": dec_wih,
            "projt": np.ascontiguousarray(projt),
            "enc_oh": _onehot_stream(C_idx[sl]),
            "dec_oh": _onehot_stream(Etgt[sl]),
        })

    res = bass_utils.run_bass_kernel_spmd(
        nc, in_maps, core_ids=list(range(NCORES)), trace=trace,
        trace_cores=[0] if trace else None)

    # ---- host-side loss assembly (float64) ----
    proj_b = np.asarray(inputs["proj_b"], np.float64)
    nll = np.empty((B, n_steps), np.float64)
    for c in range(NCORES):
        # logits[v, t*32+j] -> [n_steps, BS, V]
        lg = (np.asarray(res.results[c]["logits"], np.float64)
              .reshape(V, n_steps, BS).transpose(1, 2, 0))
        lg = lg / WSCALE + proj_b[None, None, :]
        mx = lg.max(axis=2, keepdims=True)
        lse = np.log(np.exp(lg - mx).sum(axis=2)) + mx[:, :, 0]  # [S, BS]
        tgt = Etgt[c * BS:(c + 1) * BS]                           # [BS, S]
        tl = np.take_along_axis(lg, tgt.T[:, :, None], axis=2)[:, :, 0]
        nll[c * BS:(c + 1) * BS] = (lse - tl).T
    mask = (Etgt != 0).astype(np.float64)         # [B, S]
    num = (nll * mask).sum(axis=0)
    cnt = mask.sum(axis=0)
    step_loss = np.where(cnt > 0, num / np.maximum(cnt, 1.0), 0.0)
    total = np.float32(step_loss.sum())
    return total, res


def kernel(**inputs) -> np.ndarray:
    total, _ = _run(inputs, n_steps=S,
                    trace=bool(int(os.environ.get("LSTM_TRACE", "0"))))
    return np.float32(total)
